# revision 15
# baseline (speedup 1.0000x reference)
"""Bidirectional quantized RNN (fake-quant int8 weights/acts) on 8 trn2 cores.

Sharding: core c handles direction d=c//4 (0=fwd, 1=bwd on time-reversed
input) and batch quarter q=c%4 (4 of 16 batch elements). Each core:
  Phase A: quantize its x slice to integers j=round(127*clip(x,-1,1)) and
           compute XI[n,t,b] = sum_i j[i,t,b]*k_ri[i,n] (+ b[n]/c_s) with
           bf16-integer matmuls (exact in fp32 PSUM), stored in SBUF.
  Phase B: 2048-step recurrence in transposed layout:
           gate_int = XI_t + m_t @ k_rh   (exact integers)
           t = tanh(c_s * gate_int); m_{t+1} = round(127*t); h = m/127.
All integer values |.| <= 127 are exact in bf16; all integer sums < 2^23
are exact in fp32 PSUM, so the only deviation from the fp32 reference is
tanh LUT precision and scale-application rounding (~1e-6), which the
quantized recurrence tolerates (divergence saturates at relL2 ~ 0.007).
"""
import os
from contextlib import ExitStack

import numpy as np
import ml_dtypes

import concourse.bass as bass
import concourse.bacc as bacc
import concourse.tile as tile
import concourse.mybir as mybir
from concourse.bass_utils import run_bass_kernel_spmd

SEQ, BATCH, IN, HID = 2048, 16, 512, 512
QMAX = np.float32(127.0)
C_RND = float(np.float32(12582912.0))  # 1.5 * 2^23: x+C-C == round-half-even(x)
F32 = mybir.dt.float32
BF16 = mybir.dt.bfloat16
AOP = mybir.AluOpType
ACTF = mybir.ActivationFunctionType

_cache = {}


def _build(seq, b_per_core, tb):
    """Build the single SPMD program (same for all 8 cores)."""
    nc = bacc.Bacc("TRN2")
    x_p = nc.declare_dram_parameter("x", [seq, b_per_core, IN], F32, isOutput=False)
    # all bf16 constants packed: wri 4x512 | wrh 4x512 | ident 128  (per partition)
    cb_p = nc.declare_dram_parameter("cb", [128, 4 * HID + 4 * HID + 128], BF16, isOutput=False)
    # all f32 constants packed: biasc 4 | scale 1
    cf_p = nc.declare_dram_parameter("cf", [128, 5], F32, isOutput=False)
    out_p = nc.declare_dram_parameter("out", [seq, b_per_core, HID], F32, isOutput=True)

    nblk = seq // tb
    with TileCtx(nc) as tc, ExitStack() as ctx:
        const = ctx.enter_context(tc.tile_pool(name="const", bufs=1))
        cb_sb = const.tile([128, 4 * HID + 4 * HID + 128], BF16, tag="cb")
        nc.gpsimd.dma_start(cb_sb[:], cb_p[:])
        cf_sb = const.tile([128, 5], F32, tag="cf")
        nc.gpsimd.dma_start(cf_sb[:], cf_p[:])
        # Warm up ACT function tables early: walrus prepends a table-load
        # pseudo to the first activation of each set, which eats a wait slot.
        warm = const.tile([128, 2], F32, tag="warm")
        nc.scalar.activation(warm[:, 0:1], cf_sb[:, 4:5], ACTF.Tanh)
        nc.scalar.activation(warm[:, 1:2], cf_sb[:, 4:5], ACTF.Identity)

        wri_sb = cb_sb[:, :8 * HID].rearrange("p (x n) -> p x n", x=8)  # [128, 8, 512]
        wrh_sb = wri_sb
        ident_sb = cb_sb[:, 8 * HID:8 * HID + 128]
        biasc_sb = cf_sb
        scale_sb = cf_sb
        # XI table, resident in SBUF for the whole kernel: [p, t, nchunk, b]
        xi_sb = const.tile([128, seq, 4, b_per_core], F32, tag="xi")

        # ---------------- Phase A: XI = j @ k_ri + bias/c_s ----------------
        # x loaded in natural row layout (contiguous, SWDGE-ok), quantized to
        # integers on DVE, transposed to [i, (t,b)] via PE, then matmul'd.
        tg = 32  # timesteps per 128-row group (32 t x 4 b)
        ngrp = seq // tg
        pA = ctx.enter_context(tc.tile_pool(name="pA", bufs=3))
        pAj = ctx.enter_context(tc.tile_pool(name="pAj", bufs=2))
        psT = ctx.enter_context(tc.tile_pool(name="psT", bufs=2, space="PSUM"))
        psA = ctx.enter_context(tc.tile_pool(name="psA", bufs=2, space="PSUM"))
        if True:
            for g in range(ngrp):
                xn = pA.tile([128, IN], F32, tag="xn")
                src_ap = x_p[g * tg:(g + 1) * tg].rearrange("t b i -> (t b) i")
                nc.gpsimd.dma_start(xn[:], src_ap)
                y = pA.tile([128, IN], F32, tag="y")
                nc.vector.tensor_scalar(y[:], xn[:], 127.0, C_RND, AOP.mult, AOP.add)
                z = pA.tile([128, IN], F32, tag="z")
                nc.vector.tensor_scalar(z[:], y[:], C_RND, -127.0, AOP.subtract, AOP.max)
                jn = pA.tile([128, IN], BF16, tag="jn")
                nc.vector.tensor_scalar(jn[:], z[:], 127.0, None, AOP.min)
                j_tiles = []
                for ic in range(4):
                    pst = psT.tile([128, 128], BF16, tag="pst")
                    nc.tensor.transpose(pst[:], jn[:, ic * 128:(ic + 1) * 128], ident_sb)
                    jt = pAj.tile([128, 128], BF16, tag=f"j{ic}")
                    nc.vector.tensor_copy(jt[:], pst[:])
                    j_tiles.append(jt)
                for nck in range(4):
                    ps = psA.tile([128, tg, b_per_core], F32, tag="psA")
                    for ic in range(4):
                        nc.tensor.matmul(
                            ps[:].rearrange("p t b -> p (t b)"),
                            wri_sb[:, ic, nck * 128:(nck + 1) * 128],
                            j_tiles[ic][:],
                            start=(ic == 0), stop=(ic == 3),
                        )
                    dst = xi_sb[:, g * tg:(g + 1) * tg, nck, :]
                    nc.scalar.activation(
                        dst, ps[:],
                        ACTF.Identity, bias=biasc_sb[:, nck:nck + 1], scale=1.0,
                    )

        # ---------------- Phase B: the recurrence ----------------
        pBm = ctx.enter_context(tc.tile_pool(name="pBm", bufs=3))
        pBs = ctx.enter_context(tc.tile_pool(name="pBs", bufs=3))
        pBh = ctx.enter_context(tc.tile_pool(name="pBh", bufs=4))
        psB = ctx.enter_context(tc.tile_pool(name="psB", bufs=2, space="PSUM"))
        if True:
            m_prev = pBm.tile([128, 4, b_per_core], BF16, tag="m")
            nc.vector.memset(m_prev[:], 0.0)
            for t in range(seq):
                gate = psB.tile([128, 4, b_per_core], F32, tag="gate")
                for nck in range(4):
                    for kc in range(4):
                        nc.tensor.matmul(
                            gate[:, nck, :],
                            wrh_sb[:, 4 + kc, nck * 128:(nck + 1) * 128],
                            m_prev[:, kc, :],
                            start=(kc == 0), stop=(kc == 3),
                        )
                p2 = psB.tile([128, 4, b_per_core], F32, tag="p2")
                nc.vector.tensor_add(p2[:], gate[:], xi_sb[:, t, :, :])
                th = pBs.tile([128, 4, b_per_core], F32, tag="th")
                nc.scalar.activation(th[:], p2[:], ACTF.Tanh, scale=scale_sb[:, 4:5])
                y = pBs.tile([128, 4, b_per_core], F32, tag="y")
                nc.vector.tensor_scalar(y[:], th[:], 127.0, C_RND, AOP.mult, AOP.add)
                m_prev = pBm.tile([128, 4, b_per_core], BF16, tag="m")
                nc.vector.tensor_scalar(m_prev[:], y[:], C_RND, None, AOP.subtract)
                h = pBh.tile([128, b_per_core, 4], F32, tag="h")
                nc.vector.tensor_scalar(
                    h[:].rearrange("p b c -> p c b"), y[:],
                    C_RND, 1.0 / 127.0, AOP.subtract, AOP.mult,
                )
                dst = out_p[t].rearrange("b (c p) -> p (b c)", p=128)
                nc.sync.dma_start(dst, h[:].rearrange("p b c -> p (b c)"))
    nc.compile()
    return nc


def TileCtx(nc):
    return tile.TileContext(nc)


def _host_prep(inputs, seq):
    """Per-direction weight quantization + per-core input maps."""
    x = np.ascontiguousarray(inputs["inputs"], dtype=np.float32)
    in_maps = []
    meta = []
    for d, (wri, wrh, b) in enumerate([
        (inputs["w_ri_f"], inputs["w_rh_f"], inputs["b_f"]),
        (inputs["w_ri_b"], inputs["w_rh_b"], inputs["b_b"]),
    ]):
        wri = np.asarray(wri, np.float32); wrh = np.asarray(wrh, np.float32)
        b = np.asarray(b, np.float32)
        threshold = np.float32(max(np.abs(wri).max(), np.abs(wrh).max()))
        s = np.float32(threshold / QMAX)
        k_ri = np.clip(np.round(wri / s), -QMAX, QMAX)
        k_rh = np.clip(np.round(wrh / s), -QMAX, QMAX)
        c_s = np.float32(np.float64(s) / 127.0)
        biasc = (b.astype(np.float64) / np.float64(c_s)).astype(np.float32)
        kri_b = k_ri.astype(ml_dtypes.bfloat16).reshape(4, 128, 512)
        krh_b = k_rh.astype(ml_dtypes.bfloat16).reshape(4, 128, 512)
        cb = np.concatenate(
            [kri_b.transpose(1, 0, 2).reshape(128, 2048),
             krh_b.transpose(1, 0, 2).reshape(128, 2048),
             np.eye(128, dtype=ml_dtypes.bfloat16)], axis=1)
        cf = np.concatenate(
            [biasc.reshape(4, 128).T, np.full((128, 1), c_s, np.float32)], axis=1)
        meta.append((np.ascontiguousarray(cb), np.ascontiguousarray(cf)))
    xs = [x[:seq], x[:seq][::-1]]
    for core in range(8):
        d, q = core // 4, core % 4
        cb, cf = meta[d]
        in_maps.append({
            "x": np.ascontiguousarray(xs[d][:, 4 * q:4 * q + 4, :]),
            "cb": cb, "cf": cf,
        })
    return in_maps


def _run(inputs, seq=SEQ, tb=128, trace=False):
    key = (seq, tb)
    if key not in _cache:
        _cache[key] = _build(seq, 4, tb)
    nc = _cache[key]
    in_maps = _host_prep(inputs, seq)
    res = run_bass_kernel_spmd(nc, in_maps, core_ids=list(range(8)), trace=trace)
    out = np.empty((seq, BATCH, 2 * HID), np.float32)
    for core in range(8):
        d, q = core // 4, core % 4
        o = res.results[core]["out"]
        if d == 0:
            out[:, 4 * q:4 * q + 4, :HID] = o
        else:
            out[:, 4 * q:4 * q + 4, HID:] = o[::-1]
    return out, res


def kernel(**inputs):
    out, _ = _run(inputs)
    return out
